# revision 34
# baseline (speedup 1.0000x reference)
"""ChannelPatchEmbed kernel for Trainium2 (8 NeuronCores, batch-parallel).

Computation: concat 8 single-feature channels -> each 512x512 image goes
through the SAME 1->96 conv (4x4 patches, stride 4) + bias.
Output: [8, 768, 128, 128] f32.

Strategy per core (1 batch sample per core):
  - The conv is a GEMM: for each patch, contract its 16 pixels against
    W[96, 16].  We pack all 8 channels x 16 patch-pixels onto the
    128-partition contraction dim (K = (c, i, j) = 8*4*4 = 128) and use a
    host-precomputed block-diagonal stationary matrix S so that one K=128
    matmul computes 16 output channels for all 8 input channels at once.
    6 such "oc chunk" matmuls cover all 96 output channels.
  - Input rows are DMA'd contiguously; the intra-row shift j is baked into
    the DMA (partition (c,i,j) holds image rows of channel c, row-offset i,
    pre-shifted left by j).  The matmul rhs then reads with a uniform
    stride-4 access pattern.
  - Bias is fused into the PSUM->SBUF eviction (ACT/DVE alternating).
"""

import sys

import numpy as np

if "/opt/trn_rl_repo" not in sys.path:
    sys.path.insert(0, "/opt/trn_rl_repo")

import concourse.bacc as bacc
import concourse.bass as bass
import concourse.mybir as mybir
import concourse.tile as tile
from concourse.bass_utils import run_bass_kernel_spmd

F32 = mybir.dt.float32

N_CORES = 8
C = 8            # input channels per sample (3 rgb + 4 hs + 1 dem)
H = 512          # image height/width
PATCH = 4
HP = H // PATCH  # 128 patches per side
EMBED = 96
CHUNKS = 6       # 96 output channels in chunks of 16
OCP = 16         # output channels per chunk
HPAD = 516       # padded image rows (see kernel())
HB = 16          # patch-rows per block
NBLK = HP // HB  # 8 blocks
WIN = 4          # windows per block (each window = 4 patch rows = N=512 cols)

_NC_CACHE = None


def _build_nc(reps=1):
    # detect_race_conditions=False: the sim race detector resolves SBUF APs to
    # a flat base+partition*row_bytes address model, which false-positives on
    # any concurrently-written partition-strided tiles (e.g. double-buffered
    # block N / block N+1 input loads in disjoint pool slots).
    # Bacc (not plain Bass): its compile() pipeline legalizes sync waits to
    # the HW limit of 1 per instruction (generate_event_semaphores) and moves
    # matmul waits onto ldweights.
    # reps>1 repeats the whole kernel body (identical work) for differential
    # wall-clock timing on hardware.
    nc = bacc.Bacc("TRN2", target_bir_lowering=False, detect_race_conditions=False)
    # x is host-padded to 516 rows so the j-shifted full-512 row reads stay
    # in-bounds (they read up to 3 elems past a row end, and up to 4 rows +
    # 3 elems past the last image row of a channel).
    x = nc.dram_tensor("x", [C, HPAD, H], F32, kind="ExternalInput")
    s = nc.dram_tensor("s", [CHUNKS, 128, 128], F32, kind="ExternalInput")
    bias = nc.dram_tensor("bias", [128, 128], F32, kind="ExternalInput")
    y = nc.dram_tensor("y", [C * EMBED, HP, HP], F32, kind="ExternalOutput")

    y_v = y.rearrange("(c oc) h w -> c oc (h w)", c=C)  # [8, 96, 16384]

    with tile.TileContext(nc) as tc:
        with (
            tc.tile_pool(name="const", bufs=1) as const_pool,
            tc.tile_pool(name="rin", bufs=2) as r_pool,
            tc.tile_pool(name="stage", bufs=8) as stage_pool,
            tc.tile_pool(name="psum", bufs=8, space="PSUM") as psum_pool,
        ):
            # Pad so every subsequent tile is 512 B-aligned: the framework's
            # const-scalar region ends at +128 B, and the race detector (and
            # SDMA's sub-512B RMW path) works on 512 B granules — cross-tensor
            # granule sharing between DMA writers would be flagged as a race.
            _align_pad = const_pool.tile([128, 96], F32, tag="align_pad")
            # Stationary block-diag weights: s_sb[p, chunk*128 + m]
            s_sb = const_pool.tile([128, CHUNKS * 128], F32)
            nc.sync.dma_start(
                out=s_sb[:].rearrange("p (k m) -> p k m", k=CHUNKS),
                in_=s.rearrange("k p m -> p k m"),
            )
            # Bias: bias_sb[p, chunk] (padded to 512 B/partition so the DMA's
            # sub-512B RMW write can't share a granule with the next tile)
            bias_sb = const_pool.tile([128, 128], F32)
            nc.sync.dma_start(out=bias_sb[:], in_=bias[:])

            for blk in range(NBLK * reps):
                blk = blk % NBLK
                hp0 = HB * blk
                r0 = PATCH * hp0  # first image row of this block

                # R: partition p = 8*(4i+j) + c holds, for each of the
                # block's 16 patch-rows hl, image row r0+4hl+i of channel c
                # shifted left by j (so free pos hl*512 + m = x[c, r0+4hl+i, m+j]).
                # (i,j)-major so each DMA writes a CONTIGUOUS 8-partition
                # slice — the sim's shadow memory mis-tracks partition-strided
                # DMA writes.
                R = r_pool.tile([128, HB * H], F32)
                xf = x.rearrange("c r m -> c (r m)")  # [8, HPAD*512]
                for i in range(PATCH):
                    for j in range(PATCH):
                        # rows r0+4*hl+i of every channel, shifted left by j.
                        # Full 512-elem chunks: reads cross row ends by up to
                        # j elems (harmless junk, those positions are never
                        # consumed by the matmul; host pad keeps it in-bounds).
                        off = (r0 + i) * H + j
                        src = xf[:, off : off + HB * PATCH * H].rearrange(
                            "c (hl m) -> c hl m", m=PATCH * H
                        )[:, :, :H]  # [8, 16, 512]
                        g = C * (PATCH * i + j)
                        dst = R[g : g + C].rearrange(
                            "c (hl m) -> c hl m", m=H
                        )  # [8, 16, 512]
                        nc.sync.dma_start(out=dst, in_=src)

                for chunk in range(CHUNKS):
                    lhsT = s_sb[:, chunk * 128 : (chunk + 1) * 128]
                    stg = stage_pool.tile([128, WIN * 512], F32)
                    for w in range(WIN):
                        ps = psum_pool.tile([128, 512], F32)
                        rhs = R[:, w * 2048 : (w + 1) * 2048 : PATCH]  # [128, 512]
                        nc.tensor.matmul(ps[:], lhsT, rhs, start=True, stop=True)
                        out_sl = stg[:, w * 512 : (w + 1) * 512]
                        if w % 2 == 0:
                            nc.scalar.activation(
                                out_sl,
                                ps[:],
                                mybir.ActivationFunctionType.Identity,
                                bias=bias_sb[:, chunk : chunk + 1],
                            )
                        else:
                            nc.vector.tensor_scalar_add(
                                out_sl, ps[:], bias_sb[:, chunk : chunk + 1]
                            )
                    # stg partitions 16c..16c+16 -> y[96c + 16*chunk + o, hp0:hp0+16, :]
                    # One DMA per channel c: contiguous-partition SBUF reads
                    # (partition-dim splits in DMA APs break the sim's shadow
                    # memory tracking).
                    for c in range(C):
                        # nc.scalar = the second HWDGE ring (qActDynamicHW):
                        # input loads go on the SP ring, stores on the ACT
                        # ring, so the two directions don't serialize on one
                        # descriptor ring.
                        nc.scalar.dma_start(
                            out=y_v[
                                c,
                                chunk * OCP : (chunk + 1) * OCP,
                                hp0 * HP : (hp0 + HB) * HP,
                            ],
                            in_=stg[c * OCP : (c + 1) * OCP],
                        )
    nc.compile()
    return nc


def _get_nc():
    global _NC_CACHE
    if _NC_CACHE is None:
        _NC_CACHE = _build_nc()
    return _NC_CACHE


def _host_prep(W, b):
    # Block-diagonal stationary: S[chunk, 8*(4i+j)+c, 16c+o] = W[16*chunk+o, 0, i, j]
    W6 = np.ascontiguousarray(W, dtype=np.float32).reshape(CHUNKS, OCP, 16)
    Wr = W6.transpose(0, 2, 1)  # [chunk, kij, o]
    S = np.zeros((CHUNKS, 128, 128), np.float32)
    for c in range(C):
        S[:, c::C, OCP * c : OCP * c + 16] = Wr
    bias_mat = np.tile(
        np.asarray(b, dtype=np.float32).reshape(CHUNKS, 1, OCP), (1, C, 1)
    ).reshape(CHUNKS, 128)
    # bias_pad[p, k] = bias for partition p, chunk k (padded to [128, 128])
    bias_pad = np.zeros((128, 128), np.float32)
    bias_pad[:, :CHUNKS] = bias_mat.T
    return S, bias_pad


def kernel(rgb, hs, dem, W, b):
    x_pad = np.zeros((N_CORES, C, HPAD, H), np.float32)
    x_pad[:, :3, :H] = np.asarray(rgb)
    x_pad[:, 3:7, :H] = np.asarray(hs)
    x_pad[:, 7:, :H] = np.asarray(dem)
    S, bias_mat = _host_prep(W, b)

    nc = _get_nc()
    in_maps = [
        {"x": x_pad[core], "s": S, "bias": bias_mat} for core in range(N_CORES)
    ]
    res = run_bass_kernel_spmd(nc, in_maps, list(range(N_CORES)))
    return np.stack([res.results[core]["y"] for core in range(N_CORES)], axis=0)
